# revision 20
# baseline (speedup 1.0000x reference)
"""GQA (grouped-query attention) Trainium2 kernel, 8-core SPMD, fp8-accelerated.

Problem: B=4, T=2048, d_model=2048, 32 Q heads, 8 KV heads, d_k=64, causal.
Sharding: core = (batch b, half-of-KV-heads h): 8 cores = 4 batches x 2 halves.
Each core computes its 4 KV heads (16 Q heads) for its batch and the partial
output o_half @ Wo_half (row-parallel Wo); host sums the two halves per batch
and adds bo.

fp8 plan (validated numerically on CPU):
  - Q/K projections run in fp8 e4m3 with DoubleRow perf mode (2 k-tiles per
    matmul = 2x PE throughput). x scaled 2^4, W scaled 2^12 on host.
  - q,k stay fp8 (scale 2^6) for the score matmuls (same PE speed as bf16,
    K=64; fp8 halves SBUF traffic and kills the q alignment copies via a
    4x-replicated kT so lhsT/rhs partition bases always match).
  - exp activation scale folds the 2^12 away; p is written as fp8 into
    [128,2,512] pair tiles = the DoubleRow rhs for the PV matmuls (2 key
    chunks per PV instruction = 2x).
  - V projection and O projection stay bf16: their elementwise errors hit
    early query rows (softmax over few tokens -> no error dilution).
    Query tile 0 (rows 0..511) also keeps a bf16 PV path for that reason.
  - v8 carries a 2^6 scale with a 64.0 "ones" column so numerator and
    denominator scale cancel in the softmax division.
  - Causal masking: tile 0 uses the baseline sliding wmask; tiles 1-3 use
    width-trimmed exps into persistently-zeroed diagonal pair tiles plus a
    [128,128] triangle mask multiply per diagonal chunk.
  - V-proj streams x bf16 chunks from HBM (x is only SBUF-resident in fp8).
"""

import numpy as np
import ml_dtypes
from contextlib import ExitStack

B, T, D = 4, 2048, 2048
NKV, NREP, DK = 8, 4, 64
HALF_KV = 4                  # kv heads per core
NQH = HALF_KV * NREP         # 16 q heads per core
QD = NQH * DK                # 1024 q dims per core
KVD = HALF_KV * DK           # 256 kv dims per core
NCORES = 8
CD = D // 128                # 16 contraction chunks over d_model
CT = T // 128                # 16 token chunks of 128
TQ = 512                     # query tile width
NTQ = T // TQ                # 4 query tiles
SCALE = 1.0 / np.sqrt(DK)
DKP = 80                     # padded v row (16B-aligned DoubleRow stride)

SX, SW, SQ, SV = 4, 12, 6, 6          # power-of-2 fp8 scale exponents
QCONV = 2.0 ** (SQ - SX - SW)          # psum -> q8/k8 scale
ES = SCALE * 2.0 ** (-SQ - SQ)         # exp activation scale

BF16 = ml_dtypes.bfloat16
F8 = ml_dtypes.float8_e4m3

_cache = {}


def _body(ctx, tc, aps):
    import concourse.mybir as mybir
    from concourse.bass import ts, ds

    nc = tc.nc
    f32 = mybir.dt.float32
    bf16 = mybir.dt.bfloat16
    f8 = mybir.dt.float8e4
    EXP = mybir.ActivationFunctionType.Exp
    DR = mybir.MatmulPerfMode.DoubleRow
    MUL = mybir.AluOpType.mult
    ADD = mybir.AluOpType.add

    xT8, xT16, Wq8, bq6, Wk8, bk6, Wv, bv, Wo, out = (
        aps["xT8"], aps["xT16"], aps["Wq8"], aps["bq6"], aps["Wk8"],
        aps["bk6"], aps["Wv"], aps["bv"], aps["Wo"], aps["out"])

    # ---- pools ----------------------------------------------------------
    rp = ctx.enter_context(tc.tile_pool(name="res", bufs=1))
    xp = ctx.enter_context(tc.tile_pool(name="x16", bufs=5))
    qp = ctx.enter_context(tc.tile_pool(name="qt", bufs=2))
    op = ctx.enter_context(tc.tile_pool(name="ot", bufs=2))
    ptp = ctx.enter_context(tc.tile_pool(name="pt", bufs=4))
    sp = ctx.enter_context(tc.tile_pool(name="sm", bufs=3))
    dvp = ctx.enter_context(tc.tile_pool(name="dv", bufs=2))
    wp = ctx.enter_context(tc.tile_pool(name="wk", bufs=2))
    pp2 = ctx.enter_context(tc.tile_pool(name="ps2", bufs=2, space="PSUM"))
    ppj = ctx.enter_context(tc.tile_pool(name="psj", bufs=2, space="PSUM"))
    po = ctx.enter_context(tc.tile_pool(name="po", bufs=2, space="PSUM"))

    # ---- resident tiles -------------------------------------------------
    xT8_sb = rp.tile([128, CD, T], f8, tag="xT8")            # 32 KiB/part
    Wq8_sb = rp.tile([128, CD, QD], f8, tag="Wq8")           # 16 KiB/part
    Wo_sb = rp.tile([128, QD // 128, D], bf16, tag="Wo")     # 32 KiB/part
    Wk8_sb = rp.tile([128, CD, KVD], f8, tag="Wk8")          # 4 KiB/part
    Wv_sb = rp.tile([128, CD, KVD], bf16, tag="Wv")          # 8 KiB/part
    kT8d = rp.tile([128, HALF_KV, T], f8, tag="kT8d")        # 8 KiB/part
    v8_sb = rp.tile([128, CT, HALF_KV, DKP], f8, tag="v8")   # 5 KiB/part
    v16_sb = rp.tile([128, 4, HALF_KV, DKP], bf16, tag="v16")  # 2.5 KiB/part
    bq6_sb = rp.tile([128, QD // 128], f32, tag="bq6")
    bk6_sb = rp.tile([128, KVD // 128], f32, tag="bk6")
    bv_sb = rp.tile([1, KVD], bf16, tag="bv")
    ones_b = rp.tile([1, 128], bf16, tag="ones_b")
    tri16 = rp.tile([128, 128], bf16, tag="tri16")
    tri8 = rp.tile([128, 128], f8, tag="tri8")
    # dedicated diagonal fp8 pair tiles: [h parity][pair] with persistent
    # zeros in the causally-dead column ranges (exp never writes there)
    dg8 = [[rp.tile([128, 2, TQ], f8, tag=f"dg{s}{pi}", name=f"dg{s}{pi}")
            for pi in range(2)] for s in range(2)]
    # dedicated bf16 tiles for tile 0's trimmed-exp diagonal chunks
    dg16 = [[rp.tile([128, TQ], bf16, tag=f"dh{s}{ck}", name=f"dh{s}{ck}")
             for ck in range(4)] for s in range(2)]

    # DMA order matters at startup: V-proj/K-proj window-0 inputs first
    # (window-major xT8 pieces), weights needed later go last.
    for c in range(CD):
        nc.sync.dma_start(Wv_sb[:, c, :], Wv[c * 128:(c + 1) * 128, :])
    nc.sync.dma_start(bv_sb[:, :], bv[:, :])
    for n in range(T // TQ):
        for c in range(CD):
            nc.sync.dma_start(xT8_sb[:, c, ts(n, TQ)],
                              xT8[c * 128:(c + 1) * 128, ts(n, TQ)])
    for c in range(CD):
        nc.sync.dma_start(Wk8_sb[:, c, :], Wk8[c * 128:(c + 1) * 128, :])
    for c in range(KVD // 128):
        nc.sync.dma_start(bk6_sb[:, c:c + 1], bk6[c, :].unsqueeze(-1))

    nc.vector.memset(ones_b[:, :], 1.0)
    nc.vector.memset(v8_sb[:, :, :, DK:DK + 1], float(2 ** SV))
    nc.vector.memset(v16_sb[:, :, :, DK:DK + 1], 1.0)
    # tri[p, c] = 1 where c >= p (upper triangle kept)
    nc.vector.memset(tri16[:, :], 1.0)
    nc.gpsimd.affine_select(
        out=tri16[:, :], in_=tri16[:, :],
        compare_op=mybir.AluOpType.is_ge, fill=0.0,
        base=0, pattern=[[1, 128]], channel_multiplier=-1)
    nc.vector.tensor_copy(tri8[:, :], tri16[:, :])
    for s in range(2):
        for pi in range(2):
            nc.vector.memset(dg8[s][pi][:, :, :], 0.0)
        for ck in range(4):
            nc.vector.memset(dg16[s][ck][:, :], 0.0)

    # late-needed weights: Wq8 for the tile-0 q prologue, Wo for j>=1 fillers
    for c in range(CD):
        nc.sync.dma_start(Wq8_sb[:, c, :], Wq8[c * 128:(c + 1) * 128, :])
    for c in range(QD // 128):
        nc.sync.dma_start(bq6_sb[:, c:c + 1], bq6[c, :].unsqueeze(-1))
    for c in range(QD // 128):
        nc.sync.dma_start(Wo_sb[:, c, :], Wo[c * 128:(c + 1) * 128, :])

    # ---- K^T projection (fp8 DoubleRow) into 4x-replicated layout -------
    # kT8d plane g holds kv head g's 64 dk rows replicated on partitions
    # 0:64 and 64:128, so any q partition base finds its k at the same base.
    # Generator form: only window n=0 runs in the prologue; windows 1..3
    # become tile-0 fillers (window n is first needed by query tile j=n).
    def kproj_group(m, n):
        ps = ppj.tile([128, TQ], f32, tag="psj", name=f"k{m}_{n}")
        for c2 in range(CD // 2):
            nc.tensor.matmul(ps[:, :],
                             Wk8_sb[:, 2 * c2:2 * c2 + 2, ts(m, 128)],
                             xT8_sb[:, 2 * c2:2 * c2 + 2, ts(n, TQ)],
                             start=(c2 == 0), stop=(c2 == CD // 2 - 1),
                             perf_mode=DR)
            if c2 < CD // 2 - 1:
                yield
        nc.vector.tensor_scalar(kT8d[0:64, 2 * m, ts(n, TQ)],
                                ps[0:64, :], QCONV,
                                bk6_sb[0:64, m:m + 1], MUL, ADD)
        nc.vector.tensor_scalar(kT8d[64:128, 2 * m + 1, ts(n, TQ)],
                                ps[64:128, :], QCONV,
                                bk6_sb[64:128, m:m + 1], MUL, ADD)
        nc.sync.dma_start(kT8d[64:128, 2 * m, ts(n, TQ)],
                          kT8d[0:64, 2 * m, ts(n, TQ)])
        nc.sync.dma_start(kT8d[0:64, 2 * m + 1, ts(n, TQ)],
                          kT8d[64:128, 2 * m + 1, ts(n, TQ)])
        yield

    # ---- V projection (bf16, x streamed from HBM with manual prefetch) --
    x16_tiles = {}

    def vprefetch(mt):
        t = xp.tile([128, CD, 128], bf16, tag="x16t", name=f"x16_{mt}")
        for c in range(CD):
            nc.sync.dma_start(t[:, c, :],
                              xT16[c * 128:(c + 1) * 128, ts(mt, 128)])
        x16_tiles[mt] = t

    def vproj_group(mt):
        if mt + 3 < CT:
            vprefetch(mt + 3)
        x16t = x16_tiles.pop(mt)
        ps = ppj.tile([128, TQ], f32, tag="psj", name=f"v{mt}")
        for c in range(CD):
            nc.tensor.matmul(ps[:, 0:KVD],
                             x16t[:, c, :],
                             Wv_sb[:, c, :],
                             start=(c == 0), stop=False)
            if c > 0:
                yield
        nc.tensor.matmul(ps[:, 0:KVD], ones_b[:, :], bv_sb[:, :],
                         start=False, stop=True)
        pv = ps[:, 0:KVD].rearrange("p (h d) -> p h d", h=HALF_KV)
        nc.vector.tensor_scalar_mul(v8_sb[:, mt, :, 0:DK], pv,
                                    float(2 ** SV))
        if mt < 4:
            nc.vector.tensor_copy(v16_sb[:, mt, :, 0:DK], pv)
        yield

    # ---- pipelined per-query-tile main loop -----------------------------
    def qproj_group(jj, qT_tile, m):
        # fp8 DoubleRow Q projection: one PE matmul per next()
        ps = ppj.tile([128, TQ], f32, tag="psj", name=f"q{jj}_{m}")
        for c2 in range(CD // 2):
            nc.tensor.matmul(ps[:, :],
                             Wq8_sb[:, 2 * c2:2 * c2 + 2, ts(m, 128)],
                             xT8_sb[:, 2 * c2:2 * c2 + 2, ds(jj * TQ, TQ)],
                             start=(c2 == 0), stop=(c2 == CD // 2 - 1),
                             perf_mode=DR)
            if c2 < CD // 2 - 1:
                yield
        nc.vector.tensor_scalar(qT_tile[:, m, :], ps[:, :], QCONV,
                                bq6_sb[:, m:m + 1], MUL, ADD)
        yield

    def oproj_group(jj, oT_tile, mt, n):
        ps = ppj.tile([128, TQ], f32, tag="psj", name=f"o{jj}_{mt}_{n}")
        for c in range(QD // 128):
            nc.tensor.matmul(ps[:, :],
                             oT_tile[:, c, ts(mt, 128)],
                             Wo_sb[:, c, ts(n, TQ)],
                             start=(c == 0), stop=(c == QD // 128 - 1))
            if c < QD // 128 - 1:
                yield
        os_ = wp.tile([128, TQ], f32, tag="os", name=f"os{jj}_{mt}_{n}")
        nc.vector.tensor_copy(os_[:, :], ps[:, :])
        nc.sync.dma_start(
            out[ds(jj * TQ + mt * 128, 128), ts(n, TQ)], os_[:, :])
        yield

    def filler_stream(j, qT_tiles, oT_tiles):
        # one yield per PE matmul: tile 0 absorbs the rest of the K/V
        # projections; then O-proj of tile j-1 and q-proj of tile j+1
        if j == 0:
            for n in range(1, T // TQ):
                for m in range(KVD // 128):
                    yield from kproj_group(m, n)
            for mt in range(4, CT):
                yield from vproj_group(mt)
        if j > 0:
            for mt in range(TQ // 128):
                for n in range(D // TQ):
                    yield from oproj_group(j - 1, oT_tiles[j - 1], mt, n)
        if j < NTQ - 1:
            for m in range(QD // 128):
                yield from qproj_group(j + 1, qT_tiles[j + 1], m)

    # ---- prologue: K-proj window 0, V-proj chunks 0-3, q^T for tile 0 ---
    for m in range(KVD // 128):
        for _ in kproj_group(m, 0):
            pass
    for mt in range(3):
        vprefetch(mt)
    for mt in range(4):
        for _ in vproj_group(mt):
            pass
    qT_tiles = {}
    oT_tiles = {}
    qT_tiles[0] = qp.tile([128, QD // 128, TQ], f8, tag="qT", name="qT_t0")
    for m in range(QD // 128):
        for _ in qproj_group(0, qT_tiles[0], m):
            pass

    for j in range(NTQ):
        qT_sb = qT_tiles[j]
        oT_sb = op.tile([128, QD // 128, TQ], bf16, tag="oT")
        oT_tiles[j] = oT_sb
        if j < NTQ - 1:
            qT_tiles[j + 1] = qp.tile([128, QD // 128, TQ], f8, tag="qT",
                                      name=f"qT_t{j+1}")
        nkeep = 4 * j + 4
        npairs = nkeep // 2
        filler = filler_stream(j, qT_tiles, oT_tiles)
        n_fill = ((48 + 192) if j == 0 else 128) \
            + (64 if j < NTQ - 1 else 0) + 24
        n_cks = NQH * nkeep
        fill_acc = 0.0
        fill_rate = n_fill / n_cks

        def fill(k):
            for _ in range(k):
                if next(filler, "done") == "done":
                    break

        for hq in range(NQH):
            kv = hq // NREP
            qb = (hq % 2) * 64
            qsl = qT_sb[qb:qb + 64, hq // 2, :]
            o65 = po.tile([65, TQ], f32, tag="o65")
            if j == 0:
                # bf16 PV path for the early rows (no fp8 error dilution);
                # trimmed exps into persistently-zeroed tiles + triangle mask
                pTs = {}
                for ck in range(4):
                    if ck % 2 == 0:
                        ss = pp2.tile([128, 2, TQ], f32, tag="ss",
                                      name=f"s{j}_{hq}_{ck}")
                    nc.tensor.matmul(ss[:, ck % 2, :],
                                     kT8d[qb:qb + 64, kv, ts(ck, 128)],
                                     qsl[:, :], start=True, stop=True)
                    pT = dg16[hq % 2][ck]
                    nc.scalar.activation(
                        pT[:, ds(128 * ck, TQ - 128 * ck)],
                        ss[:, ck % 2, ds(128 * ck, TQ - 128 * ck)],
                        EXP, scale=ES)
                    nc.vector.tensor_mul(pT[:, ds(128 * ck, 128)],
                                         pT[:, ds(128 * ck, 128)],
                                         tri16[:, :])
                    pTs[ck] = pT
                    if ck > 0:
                        nc.tensor.matmul(o65[:, :],
                                         v16_sb[:, ck - 1, kv, 0:DK + 1],
                                         pTs[ck - 1][:, :],
                                         start=(ck - 1 == 0), stop=False)
                        del pTs[ck - 1]
                    fill_acc += fill_rate
                    k = int(fill_acc)
                    fill_acc -= k
                    fill(k)
                nc.tensor.matmul(o65[:, :], v16_sb[:, 3, kv, 0:DK + 1],
                                 pTs[3][:, :], start=False, stop=True)
                del pTs[3]
            else:
                # fp8 pair path: DoubleRow PV over [128,2,512] pair tiles.
                # Diagonal pairs are issued FIRST so their DVE triangle
                # masks clear well before their PV consumes them.
                prev = None
                prev_pk = None
                first = True
                for pk in [2 * j, 2 * j + 1] + list(range(2 * j)):
                    ss = pp2.tile([128, 2, TQ], f32, tag="ss",
                                  name=f"s{j}_{hq}_{pk}")
                    for i in range(2):
                        ck = 2 * pk + i
                        nc.tensor.matmul(ss[:, i, :],
                                         kT8d[qb:qb + 64, kv, ts(ck, 128)],
                                         qsl[:, :], start=True, stop=True)
                        fill_acc += fill_rate
                        k = int(fill_acc)
                        fill_acc -= k
                        fill(k)
                    di0 = 2 * pk - 4 * j
                    if di0 < 0:
                        pT8 = ptp.tile([128, 2, TQ], f8, tag="pT8",
                                       name=f"p{j}_{hq}_{pk}")
                        nc.scalar.activation(pT8[:, :, :], ss[:, :, :], EXP,
                                             scale=ES)
                    else:
                        pT8 = dg8[hq % 2][pk - 2 * j]
                        for i in range(2):
                            di = di0 + i
                            nc.scalar.activation(
                                pT8[:, i, ds(128 * di, TQ - 128 * di)],
                                ss[:, i, ds(128 * di, TQ - 128 * di)],
                                EXP, scale=ES)
                            nc.vector.tensor_mul(
                                pT8[:, i, ds(128 * di, 128)],
                                pT8[:, i, ds(128 * di, 128)], tri8[:, :])
                    if prev is not None:
                        nc.tensor.matmul(
                            o65[:, :],
                            v8_sb[:, 2 * prev_pk:2 * prev_pk + 2, kv,
                                  0:DK + 1],
                            prev[:, :, :],
                            start=first, stop=False, perf_mode=DR)
                        first = False
                    prev, prev_pk = pT8, pk
                nc.tensor.matmul(
                    o65[:, :],
                    v8_sb[:, 2 * prev_pk:2 * prev_pk + 2, kv, 0:DK + 1],
                    prev[:, :, :],
                    start=first, stop=True, perf_mode=DR)
            # softmax division: 1/sums (row 64) broadcast over the 64
            # o^T rows, fused with the psum->sbuf eviction
            srow = dvp.tile([1, TQ], f32, tag="sr")
            nc.vector.tensor_copy(srow[:, :], o65[64:65, :])
            rrow = dvp.tile([1, TQ], f32, tag="rr")
            nc.vector.reciprocal_approx_fast(rrow[:, :], srow[:, :])
            bcs = dvp.tile([64, TQ], f32, tag="bc")
            nc.gpsimd.partition_broadcast(bcs[:, :], rrow[:, :])
            nc.vector.tensor_mul(
                oT_sb[qb:qb + 64, hq // 2, :],
                o65[0:64, :], bcs[:, :])
        fill(n_fill)

    # epilogue: O-projection of the last tile
    for mt in range(TQ // 128):
        for n in range(D // TQ):
            for _ in oproj_group(NTQ - 1, oT_tiles[NTQ - 1], mt, n):
                pass


def _build():
    import concourse.mybir as mybir
    import concourse.tile as tile
    from concourse import bacc

    nc = bacc.Bacc("TRN2", target_bir_lowering=False, debug=False,
                   num_devices=NCORES)
    f32, bf16, f8 = mybir.dt.float32, mybir.dt.bfloat16, mybir.dt.float8e4
    aps = {
        "xT8": nc.dram_tensor("xT8", (D, T), f8, kind="ExternalInput").ap(),
        "xT16": nc.dram_tensor("xT16", (D, T), bf16,
                               kind="ExternalInput").ap(),
        "Wq8": nc.dram_tensor("Wq8", (D, QD), f8, kind="ExternalInput").ap(),
        "bq6": nc.dram_tensor("bq6", (QD // 128, 128), f32,
                              kind="ExternalInput").ap(),
        "Wk8": nc.dram_tensor("Wk8", (D, KVD), f8, kind="ExternalInput").ap(),
        "bk6": nc.dram_tensor("bk6", (KVD // 128, 128), f32,
                              kind="ExternalInput").ap(),
        "Wv": nc.dram_tensor("Wv", (D, KVD), bf16, kind="ExternalInput").ap(),
        "bv": nc.dram_tensor("bv", (1, KVD), bf16, kind="ExternalInput").ap(),
        "Wo": nc.dram_tensor("Wo", (QD, D), bf16, kind="ExternalInput").ap(),
        "out": nc.dram_tensor("out", (T, D), f32, kind="ExternalOutput").ap(),
    }
    with tile.TileContext(nc) as tc:
        with ExitStack() as ctx:
            _body(ctx, tc, aps)
    nc.compile()
    return nc


def _get_nc():
    if "nc" not in _cache:
        _cache["nc"] = _build()
    return _cache["nc"]


def _in_maps(x, Wq, bq, Wk, bk, Wv, bv, Wo):
    x = np.asarray(x, np.float32)
    maps = []
    for core in range(NCORES):
        b, h = core // 2, core % 2
        xT = np.ascontiguousarray(np.asarray(x[b]).T)
        maps.append({
            "xT8": (xT * 2.0 ** SX).astype(F8),
            "xT16": xT.astype(BF16),
            "Wq8": (np.asarray(Wq[:, h * QD:(h + 1) * QD], np.float32)
                    * 2.0 ** SW).astype(F8),
            "bq6": (np.asarray(bq[h * QD:(h + 1) * QD], np.float32)
                    * 2.0 ** SQ).reshape(QD // 128, 128),
            "Wk8": (np.asarray(Wk[:, h * KVD:(h + 1) * KVD], np.float32)
                    * 2.0 ** SW).astype(F8),
            "bk6": (np.asarray(bk[h * KVD:(h + 1) * KVD], np.float32)
                    * 2.0 ** SQ).reshape(KVD // 128, 128),
            "Wv": np.asarray(Wv[:, h * KVD:(h + 1) * KVD],
                             np.float32).astype(BF16),
            "bv": np.asarray(bv[h * KVD:(h + 1) * KVD],
                             np.float32).reshape(1, KVD).astype(BF16),
            "Wo": np.asarray(Wo[h * QD:(h + 1) * QD, :],
                             np.float32).astype(BF16),
        })
    return maps


def kernel(x, Wq, bq, Wk, bk, Wv, bv, Wo, bo, **_):
    from concourse.bass_utils import run_bass_kernel_spmd

    in_maps = _in_maps(x, Wq, bq, Wk, bk, Wv, bv, Wo)
    nc = _get_nc()
    res = run_bass_kernel_spmd(nc, in_maps, core_ids=list(range(NCORES)))
    bo = np.asarray(bo, np.float32)
    outs = [np.asarray(res.results[c]["out"], np.float32)
            for c in range(NCORES)]
    return np.stack([outs[2 * b] + outs[2 * b + 1] + bo
                     for b in range(B)], axis=0)
